# revision 1
# baseline (speedup 1.0000x reference)
"""Trainium2 Bass kernel for nn_DifferentiableAugmentation.

The reference's straight-through estimators are numerically exact in fp32:
the gumbel-softmax hard weights are an exact one-hot (Sterbenz: for the
selected index y >= 0.5, so (1-y)+y == 1.0 exactly; for the other index
(0-y)+y == 0.0 exactly), and the relaxed-Bernoulli hard sample is exactly
0.0 or 1.0.  The whole forward therefore collapses to, per round:
pick op j = argmax(logits + gumbel); if the Bernoulli bit is 1, apply a
nearest-neighbor x-warp (shear-x or translate-x) with zero fill.

Both warps are per-row gathers along width whose source index is
round(w + const(h)), i.e. a per-row horizontal SHIFT with zero fill at the
invalid edges.  The composition of the two rounds is again a per-row shift
with a per-row valid interval.  The only remaining fp effect is the
straight-through magnitude perturbation x -> (x+m)-m (<= 1 ulp of ~2 per
round, ~2.4e-7 absolute), which we intentionally drop.

So the device kernel is pure data movement: for each group of rows with
identical (shift, valid interval) structure, one DRAM->DRAM DMA copy per
segment plus DMA zero-fills of the invalid margins from a zeros tensor.
Data parallel over batch: 256/8 = 32 images per NeuronCore.
"""

import numpy as np

from concourse import bass
from concourse.bass_utils import run_bass_kernel_spmd
import concourse.mybir as mybir

B, H, W, C = 256, 224, 224, 3
N_CORES = 8
B_LOC = B // N_CORES
_EPS = 1e-6
_f32 = np.float32


def _round_decisions(sp_weights, sp_probs, sp_magnitudes, u_gumbel, u_logistic):
    """Mirror the reference's fp32 scalar math. Returns [(applied, j, m), ...]."""
    sp_weights = np.asarray(sp_weights, _f32)
    sp_probs = np.asarray(sp_probs, _f32)
    sp_magnitudes = np.asarray(sp_magnitudes, _f32)
    u_gumbel = np.asarray(u_gumbel, _f32)
    u_logistic = np.asarray(u_logistic, _f32)
    num_ops = sp_weights.shape[0]
    rounds = []
    for i in range(num_ops):
        u = np.clip(u_gumbel[i], _f32(_EPS), _f32(1.0 - _EPS))
        g = (-np.log(-np.log(u))).astype(_f32)
        z = (sp_weights[i] + g).astype(_f32)
        j = int(np.argmax(z))
        p = np.clip(sp_probs[i, j], _f32(_EPS), _f32(1.0 - _EPS))
        ul = np.clip(u_logistic[i, j], _f32(_EPS), _f32(1.0 - _EPS))
        l = (np.log(ul) - np.log1p(-ul)).astype(_f32)
        logit = ((np.log(p) - np.log1p(-p)).astype(_f32) + l).astype(_f32)
        y = _f32(1.0) / (_f32(1.0) + np.exp(-logit).astype(_f32))
        applied = bool(np.round(y) == 1.0)
        rounds.append((applied, j, sp_magnitudes[i, j]))
    return rounds


def _warp_maps(applied, j, m):
    """Per-round (xic[H,W] int32 clipped src index, valid[H,W] bool)."""
    ww = np.arange(W, dtype=_f32)[None, :]
    if not applied:
        xi = np.broadcast_to(np.arange(W, dtype=np.int32)[None, :], (H, W))
        return xi, np.ones((H, W), dtype=bool)
    if j == 0:  # ShearX: x_src = x + (0.6*m - 0.3) * y
        mag = _f32(_f32(0.6) * m - _f32(0.3))
        yy = np.arange(H, dtype=_f32)[:, None]
        x_src = (ww + (mag * yy).astype(_f32)).astype(_f32)
    else:  # TranslateX: x_src = x - (20*m - 10)
        pix = _f32(_f32(20.0) * m - _f32(10.0))
        x_src = np.broadcast_to((np.arange(W, dtype=_f32) - pix).astype(_f32)[None, :], (H, W))
    xi = np.round(x_src).astype(np.int32)
    valid = (xi >= 0) & (xi < W)
    xic = np.clip(xi, 0, W - 1)
    return xic, valid


def _composed_map(rounds):
    """Compose the per-round warps into (src[H,W] int, mask[H,W] bool)."""
    rows = np.arange(H)[:, None]
    src = np.broadcast_to(np.arange(W)[None, :], (H, W)).copy()
    mask = np.ones((H, W), dtype=bool)
    for applied, j, m in rounds:
        xic, valid = _warp_maps(applied, j, m)
        src = src[rows, xic]
        mask = mask[rows, xic] & valid
    return src, mask


def _row_groups(src, mask):
    """Decompose the map into row groups with identical segment structure.

    Returns list of (h0, h1, segs) where segs is a tuple of (lo, hi, s):
    out[:, h0:h1, lo:hi, :] = x[:, h0:h1, lo+s:hi+s, :], zero elsewhere.
    """
    per_row = []
    for h in range(H):
        segs = []
        w = 0
        mrow = mask[h]
        srow = src[h]
        while w < W:
            if not mrow[w]:
                w += 1
                continue
            s = int(srow[w]) - w
            w2 = w + 1
            while w2 < W and mrow[w2] and int(srow[w2]) - w2 == s:
                w2 += 1
            segs.append((w, w2, s))
            w = w2
        per_row.append(tuple(segs))
    groups = []
    h = 0
    while h < H:
        h2 = h + 1
        while h2 < H and per_row[h2] == per_row[h]:
            h2 += 1
        groups.append((h, h2, per_row[h]))
        h = h2
    return groups


def _gaps(segs):
    """Complement of the copy segments in [0, W)."""
    out = []
    prev = 0
    for lo, hi, _ in segs:
        if lo > prev:
            out.append((prev, lo))
        prev = hi
    if prev < W:
        out.append((prev, W))
    return out


def _build_program(groups):
    dt = mybir.dt.float32
    nc = bass.Bass()
    xin = nc.declare_dram_parameter("xin", [B_LOC, H, W, C], dt, isOutput=False)
    zeros = nc.declare_dram_parameter("zeros", [B_LOC, H, W, C], dt, isOutput=False)
    out = nc.declare_dram_parameter("out", [B_LOC, H, W, C], dt, isOutput=True)

    with nc.Block() as block, nc.semaphore("dma_sem") as dma_sem:

        @block.sync
        def _(sync):
            n = 0
            for h0, h1, segs in groups:
                for lo, hi, s in segs:
                    sync.dma_start(
                        out=out[:, h0:h1, lo:hi, :],
                        in_=xin[:, h0:h1, lo + s : hi + s, :],
                    ).then_inc(dma_sem, 16)
                    n += 1
                for lo, hi in _gaps(segs):
                    sync.dma_start(
                        out=out[:, h0:h1, lo:hi, :],
                        in_=zeros[:, h0:h1, 0 : hi - lo, :],
                    ).then_inc(dma_sem, 16)
                    n += 1
            sync.wait_ge(dma_sem, 16 * n)

    return nc


_PROGRAM_CACHE = {}


def kernel(x, sp_weights, sp_probs, sp_magnitudes, u_gumbel, u_logistic):
    x = np.ascontiguousarray(np.asarray(x, dtype=np.float32))
    rounds = _round_decisions(sp_weights, sp_probs, sp_magnitudes, u_gumbel, u_logistic)
    src, mask = _composed_map(rounds)
    groups = _row_groups(src, mask)

    key = tuple(groups)
    nc = _PROGRAM_CACHE.get(key)
    if nc is None:
        nc = _build_program(groups)
        _PROGRAM_CACHE[key] = nc

    z = np.zeros((B_LOC, H, W, C), dtype=np.float32)
    in_maps = [
        {"xin": x[i * B_LOC : (i + 1) * B_LOC], "zeros": z} for i in range(N_CORES)
    ]
    res = run_bass_kernel_spmd(nc, in_maps, list(range(N_CORES)))
    return np.concatenate([res.results[i]["out"] for i in range(N_CORES)], axis=0)


# revision 2
# speedup vs baseline: 1.1408x; 1.1408x over previous
"""Trainium2 Bass kernel for nn_DifferentiableAugmentation.

The reference's straight-through estimators are numerically exact in fp32:
the gumbel-softmax hard weights are an exact one-hot (Sterbenz: for the
selected index y >= 0.5, so (1-y)+y == 1.0 exactly; for the other index
(0-y)+y == 0.0 exactly), and the relaxed-Bernoulli hard sample is exactly
0.0 or 1.0.  The whole forward therefore collapses to, per round:
pick op j = argmax(logits + gumbel); if the Bernoulli bit is 1, apply a
nearest-neighbor x-warp (shear-x or translate-x) with zero fill.

Both warps are per-row gathers along width whose source index is
round(w + const(h)), i.e. a per-row horizontal SHIFT with zero fill at the
invalid edges.  The composition of the two rounds is again a per-row shift
with a per-row valid interval.  The only remaining fp effect is the
straight-through magnitude perturbation x -> (x+m)-m (<= 1 ulp of ~2 per
round, ~2.4e-7 absolute), which we intentionally drop.

So the device kernel is pure data movement: for each group of rows with
identical (shift, valid interval) structure, one DRAM->DRAM DMA copy per
segment plus DMA zero-fills of the invalid margins from a zeros tensor.
Data parallel over batch: 256/8 = 32 images per NeuronCore.
"""

import numpy as np

from concourse import bass
from concourse.bass_utils import run_bass_kernel_spmd
import concourse.mybir as mybir

B, H, W, C = 256, 224, 224, 3
N_CORES = 8
B_LOC = B // N_CORES
_EPS = 1e-6
_f32 = np.float32


def _round_decisions(sp_weights, sp_probs, sp_magnitudes, u_gumbel, u_logistic):
    """Mirror the reference's fp32 scalar math. Returns [(applied, j, m), ...]."""
    sp_weights = np.asarray(sp_weights, _f32)
    sp_probs = np.asarray(sp_probs, _f32)
    sp_magnitudes = np.asarray(sp_magnitudes, _f32)
    u_gumbel = np.asarray(u_gumbel, _f32)
    u_logistic = np.asarray(u_logistic, _f32)
    num_ops = sp_weights.shape[0]
    rounds = []
    for i in range(num_ops):
        u = np.clip(u_gumbel[i], _f32(_EPS), _f32(1.0 - _EPS))
        g = (-np.log(-np.log(u))).astype(_f32)
        z = (sp_weights[i] + g).astype(_f32)
        j = int(np.argmax(z))
        p = np.clip(sp_probs[i, j], _f32(_EPS), _f32(1.0 - _EPS))
        ul = np.clip(u_logistic[i, j], _f32(_EPS), _f32(1.0 - _EPS))
        l = (np.log(ul) - np.log1p(-ul)).astype(_f32)
        logit = ((np.log(p) - np.log1p(-p)).astype(_f32) + l).astype(_f32)
        y = _f32(1.0) / (_f32(1.0) + np.exp(-logit).astype(_f32))
        applied = bool(np.round(y) == 1.0)
        rounds.append((applied, j, sp_magnitudes[i, j]))
    return rounds


def _warp_maps(applied, j, m):
    """Per-round (xic[H,W] int32 clipped src index, valid[H,W] bool)."""
    ww = np.arange(W, dtype=_f32)[None, :]
    if not applied:
        xi = np.broadcast_to(np.arange(W, dtype=np.int32)[None, :], (H, W))
        return xi, np.ones((H, W), dtype=bool)
    if j == 0:  # ShearX: x_src = x + (0.6*m - 0.3) * y
        mag = _f32(_f32(0.6) * m - _f32(0.3))
        yy = np.arange(H, dtype=_f32)[:, None]
        x_src = (ww + (mag * yy).astype(_f32)).astype(_f32)
    else:  # TranslateX: x_src = x - (20*m - 10)
        pix = _f32(_f32(20.0) * m - _f32(10.0))
        x_src = np.broadcast_to((np.arange(W, dtype=_f32) - pix).astype(_f32)[None, :], (H, W))
    xi = np.round(x_src).astype(np.int32)
    valid = (xi >= 0) & (xi < W)
    xic = np.clip(xi, 0, W - 1)
    return xic, valid


def _composed_map(rounds):
    """Compose the per-round warps into (src[H,W] int, mask[H,W] bool)."""
    rows = np.arange(H)[:, None]
    src = np.broadcast_to(np.arange(W)[None, :], (H, W)).copy()
    mask = np.ones((H, W), dtype=bool)
    for applied, j, m in rounds:
        xic, valid = _warp_maps(applied, j, m)
        src = src[rows, xic]
        mask = mask[rows, xic] & valid
    return src, mask


def _row_groups(src, mask):
    """Decompose the map into row groups with identical segment structure.

    Returns list of (h0, h1, segs) where segs is a tuple of (lo, hi, s):
    out[:, h0:h1, lo:hi, :] = x[:, h0:h1, lo+s:hi+s, :], zero elsewhere.
    """
    per_row = []
    for h in range(H):
        segs = []
        w = 0
        mrow = mask[h]
        srow = src[h]
        while w < W:
            if not mrow[w]:
                w += 1
                continue
            s = int(srow[w]) - w
            w2 = w + 1
            while w2 < W and mrow[w2] and int(srow[w2]) - w2 == s:
                w2 += 1
            segs.append((w, w2, s))
            w = w2
        per_row.append(tuple(segs))
    groups = []
    h = 0
    while h < H:
        h2 = h + 1
        while h2 < H and per_row[h2] == per_row[h]:
            h2 += 1
        groups.append((h, h2, per_row[h]))
        h = h2
    return groups


def _gaps(segs):
    """Complement of the copy segments in [0, W)."""
    out = []
    prev = 0
    for lo, hi, _ in segs:
        if lo > prev:
            out.append((prev, lo))
        prev = hi
    if prev < W:
        out.append((prev, W))
    return out


def _build_program(groups):
    """One DRAM->DRAM DMA per copy segment / zero gap, issued back-to-back on
    the SP HWDGE ring.  DMA completions rotate over multiple semaphores so no
    single semaphore accumulates a large count (<= 120 DMAs x 16 incs each);
    the kernel waits on all of them at the end."""
    from contextlib import ExitStack

    dt = mybir.dt.float32
    nc = bass.Bass()
    xin = nc.declare_dram_parameter("xin", [B_LOC, H, W, C], dt, isOutput=False)
    zeros = nc.declare_dram_parameter("zeros", [B_LOC, H, W, C], dt, isOutput=False)
    out = nc.declare_dram_parameter("out", [B_LOC, H, W, C], dt, isOutput=True)

    jobs = []
    for h0, h1, segs in groups:
        for lo, hi, s in segs:
            jobs.append(((slice(None), slice(h0, h1), slice(lo, hi), slice(None)),
                         xin,
                         (slice(None), slice(h0, h1), slice(lo + s, hi + s), slice(None))))
        for lo, hi in _gaps(segs):
            jobs.append(((slice(None), slice(h0, h1), slice(lo, hi), slice(None)),
                         zeros,
                         (slice(None), slice(h0, h1), slice(0, hi - lo), slice(None))))

    chunk = 120
    max_sems = 64
    n_sems = max(1, min(max_sems, -(-len(jobs) // chunk)))
    batch = n_sems * chunk

    with ExitStack() as stack:
        block = stack.enter_context(nc.Block())
        sems = [stack.enter_context(nc.semaphore(f"dma_sem{i}")) for i in range(n_sems)]

        @block.sync
        def _(sync):
            target = [0] * n_sems
            for b0 in range(0, len(jobs), batch):
                bjobs = jobs[b0 : b0 + batch]
                for idx, (o_sl, src_t, i_sl) in enumerate(bjobs):
                    s = idx // chunk
                    sync.dma_start(out=out[o_sl], in_=src_t[i_sl]).then_inc(sems[s], 16)
                    target[s] += 16
                for s in range(n_sems):
                    if target[s]:
                        sync.wait_ge(sems[s], target[s])
                if b0 + batch < len(jobs):
                    # absurdly fragmented map: reset sems between mega-batches
                    for s in range(n_sems):
                        if target[s]:
                            sync.sem_clear(sems[s])
                            target[s] = 0

    return nc


_PROGRAM_CACHE = {}


def kernel(x, sp_weights, sp_probs, sp_magnitudes, u_gumbel, u_logistic):
    x = np.ascontiguousarray(np.asarray(x, dtype=np.float32))
    rounds = _round_decisions(sp_weights, sp_probs, sp_magnitudes, u_gumbel, u_logistic)
    src, mask = _composed_map(rounds)
    groups = _row_groups(src, mask)

    key = tuple(groups)
    nc = _PROGRAM_CACHE.get(key)
    if nc is None:
        nc = _build_program(groups)
        _PROGRAM_CACHE[key] = nc

    z = np.zeros((B_LOC, H, W, C), dtype=np.float32)
    in_maps = [
        {"xin": x[i * B_LOC : (i + 1) * B_LOC], "zeros": z} for i in range(N_CORES)
    ]
    res = run_bass_kernel_spmd(nc, in_maps, list(range(N_CORES)))
    return np.concatenate([res.results[i]["out"] for i in range(N_CORES)], axis=0)


# revision 5
# speedup vs baseline: 1.3079x; 1.1464x over previous
"""Trainium2 Bass kernel for nn_DifferentiableAugmentation.

The reference's straight-through estimators are numerically exact in fp32:
the gumbel-softmax hard weights are an exact one-hot (Sterbenz: for the
selected index y >= 0.5, so (1-y)+y == 1.0 exactly; for the other index
(0-y)+y == 0.0 exactly), and the relaxed-Bernoulli hard sample is exactly
0.0 or 1.0.  The whole forward therefore collapses to, per round:
pick op j = argmax(logits + gumbel); if the Bernoulli bit is 1, apply a
nearest-neighbor x-warp (shear-x or translate-x) with zero fill.

Both warps are per-row gathers along width whose source index is
round(w + const(h)), i.e. a per-row horizontal SHIFT with zero fill at the
invalid edges.  The composition of the two rounds is again a per-row shift
with a per-row valid interval.  The only remaining fp effect is the
straight-through magnitude perturbation x -> (x+m)-m (<= 1 ulp of ~2 per
round, ~2.4e-7 absolute), which we intentionally drop.

So the device kernel is pure data movement: for each group of rows with
identical (shift, valid interval) structure, one DRAM->DRAM DMA copy per
segment.  Invalid margins are NOT written at all: run_bass_kernel_spmd
guarantees pre-zeroed ExternalOutput buffers (the native path zeroes them
before run_neff; the axon/PJRT path donates freshly-zeroed buffers --
bass2jax.run_bass_via_pjrt documents that kernels which don't write every
element rely on this).  Skipping the margin writes avoids 7k+ sub-512B
descriptors whose read-modify-write amplification measured ~20% of kernel
time.  Data parallel over batch: 256/8 = 32 images per NeuronCore.
"""

import numpy as np

from concourse import bass
from concourse.bass_utils import run_bass_kernel_spmd
import concourse.mybir as mybir

B, H, W, C = 256, 224, 224, 3
N_CORES = 8
B_LOC = B // N_CORES
_EPS = 1e-6
_f32 = np.float32


def _round_decisions(sp_weights, sp_probs, sp_magnitudes, u_gumbel, u_logistic):
    """Mirror the reference's fp32 scalar math. Returns [(applied, j, m), ...]."""
    sp_weights = np.asarray(sp_weights, _f32)
    sp_probs = np.asarray(sp_probs, _f32)
    sp_magnitudes = np.asarray(sp_magnitudes, _f32)
    u_gumbel = np.asarray(u_gumbel, _f32)
    u_logistic = np.asarray(u_logistic, _f32)
    num_ops = sp_weights.shape[0]
    rounds = []
    for i in range(num_ops):
        u = np.clip(u_gumbel[i], _f32(_EPS), _f32(1.0 - _EPS))
        g = (-np.log(-np.log(u))).astype(_f32)
        z = (sp_weights[i] + g).astype(_f32)
        j = int(np.argmax(z))
        p = np.clip(sp_probs[i, j], _f32(_EPS), _f32(1.0 - _EPS))
        ul = np.clip(u_logistic[i, j], _f32(_EPS), _f32(1.0 - _EPS))
        l = (np.log(ul) - np.log1p(-ul)).astype(_f32)
        logit = ((np.log(p) - np.log1p(-p)).astype(_f32) + l).astype(_f32)
        y = _f32(1.0) / (_f32(1.0) + np.exp(-logit).astype(_f32))
        applied = bool(np.round(y) == 1.0)
        rounds.append((applied, j, sp_magnitudes[i, j]))
    return rounds


def _warp_maps(applied, j, m):
    """Per-round (xic[H,W] int32 clipped src index, valid[H,W] bool)."""
    ww = np.arange(W, dtype=_f32)[None, :]
    if not applied:
        xi = np.broadcast_to(np.arange(W, dtype=np.int32)[None, :], (H, W))
        return xi, np.ones((H, W), dtype=bool)
    if j == 0:  # ShearX: x_src = x + (0.6*m - 0.3) * y
        mag = _f32(_f32(0.6) * m - _f32(0.3))
        yy = np.arange(H, dtype=_f32)[:, None]
        x_src = (ww + (mag * yy).astype(_f32)).astype(_f32)
    else:  # TranslateX: x_src = x - (20*m - 10)
        pix = _f32(_f32(20.0) * m - _f32(10.0))
        x_src = np.broadcast_to((np.arange(W, dtype=_f32) - pix).astype(_f32)[None, :], (H, W))
    xi = np.round(x_src).astype(np.int32)
    valid = (xi >= 0) & (xi < W)
    xic = np.clip(xi, 0, W - 1)
    return xic, valid


def _composed_map(rounds):
    """Compose the per-round warps into (src[H,W] int, mask[H,W] bool)."""
    rows = np.arange(H)[:, None]
    src = np.broadcast_to(np.arange(W)[None, :], (H, W)).copy()
    mask = np.ones((H, W), dtype=bool)
    for applied, j, m in rounds:
        xic, valid = _warp_maps(applied, j, m)
        src = src[rows, xic]
        mask = mask[rows, xic] & valid
    return src, mask


def _row_groups(src, mask):
    """Decompose the map into row groups with identical segment structure.

    Returns list of (h0, h1, segs) where segs is a tuple of (lo, hi, s):
    out[:, h0:h1, lo:hi, :] = x[:, h0:h1, lo+s:hi+s, :], zero elsewhere.
    """
    per_row = []
    for h in range(H):
        segs = []
        w = 0
        mrow = mask[h]
        srow = src[h]
        while w < W:
            if not mrow[w]:
                w += 1
                continue
            s = int(srow[w]) - w
            w2 = w + 1
            while w2 < W and mrow[w2] and int(srow[w2]) - w2 == s:
                w2 += 1
            segs.append((w, w2, s))
            w = w2
        per_row.append(tuple(segs))
    groups = []
    h = 0
    while h < H:
        h2 = h + 1
        while h2 < H and per_row[h2] == per_row[h]:
            h2 += 1
        groups.append((h, h2, per_row[h]))
        h = h2
    return groups


def _gaps(segs):
    """Complement of the copy segments in [0, W)."""
    out = []
    prev = 0
    for lo, hi, _ in segs:
        if lo > prev:
            out.append((prev, lo))
        prev = hi
    if prev < W:
        out.append((prev, W))
    return out


def _build_program(groups):
    """One DRAM->DRAM DMA per copy segment, issued back-to-back on the SP
    HWDGE ring.  Zero margins are never written (output buffers arrive
    pre-zeroed, see module docstring).  DMA completions rotate over multiple
    semaphores so no single semaphore accumulates a large count (<= 120 DMAs
    x 16 incs each); the kernel waits on all of them at the end."""
    from contextlib import ExitStack

    dt = mybir.dt.float32
    nc = bass.Bass()
    xin = nc.declare_dram_parameter("xin", [B_LOC, H, W, C], dt, isOutput=False)
    out = nc.declare_dram_parameter("out", [B_LOC, H, W, C], dt, isOutput=True)

    jobs = []
    for h0, h1, segs in groups:
        for lo, hi, s in segs:
            jobs.append(((slice(None), slice(h0, h1), slice(lo, hi), slice(None)),
                         xin,
                         (slice(None), slice(h0, h1), slice(lo + s, hi + s), slice(None))))

    if not jobs:
        # fully-masked output (all zeros): nothing to copy, but emit one tiny
        # DMA into an internal scratch tensor so the program is non-empty.
        scratch = nc.dram_tensor("scratch", [1, C], dt)
        with nc.Block() as block, nc.semaphore("dma_sem0") as sem:
            @block.sync
            def _(sync):
                sync.dma_start(out=scratch[:], in_=xin[0, 0, 0:1, :]).then_inc(sem, 16)
                sync.wait_ge(sem, 16)
        return nc

    chunk = 120
    max_sems = 64
    n_sems = max(1, min(max_sems, -(-len(jobs) // chunk)))
    batch = n_sems * chunk

    with ExitStack() as stack:
        block = stack.enter_context(nc.Block())
        sems = [stack.enter_context(nc.semaphore(f"dma_sem{i}")) for i in range(n_sems)]

        @block.sync
        def _(sync):
            target = [0] * n_sems
            for b0 in range(0, len(jobs), batch):
                bjobs = jobs[b0 : b0 + batch]
                for idx, (o_sl, src_t, i_sl) in enumerate(bjobs):
                    s = idx // chunk
                    sync.dma_start(out=out[o_sl], in_=src_t[i_sl]).then_inc(sems[s], 16)
                    target[s] += 16
                for s in range(n_sems):
                    if target[s]:
                        sync.wait_ge(sems[s], target[s])
                if b0 + batch < len(jobs):
                    # absurdly fragmented map: reset sems between mega-batches
                    for s in range(n_sems):
                        if target[s]:
                            sync.sem_clear(sems[s])
                            target[s] = 0

    return nc


_PROGRAM_CACHE = {}


def kernel(x, sp_weights, sp_probs, sp_magnitudes, u_gumbel, u_logistic):
    x = np.ascontiguousarray(np.asarray(x, dtype=np.float32))
    rounds = _round_decisions(sp_weights, sp_probs, sp_magnitudes, u_gumbel, u_logistic)
    src, mask = _composed_map(rounds)
    groups = _row_groups(src, mask)

    key = tuple(groups)
    nc = _PROGRAM_CACHE.get(key)
    if nc is None:
        nc = _build_program(groups)
        _PROGRAM_CACHE[key] = nc

    in_maps = [{"xin": x[i * B_LOC : (i + 1) * B_LOC]} for i in range(N_CORES)]
    res = run_bass_kernel_spmd(nc, in_maps, list(range(N_CORES)))
    return np.concatenate([res.results[i]["out"] for i in range(N_CORES)], axis=0)


# revision 7
# speedup vs baseline: 1.3490x; 1.0315x over previous
"""Trainium2 Bass kernel for nn_DifferentiableAugmentation.

The reference's straight-through estimators are numerically exact in fp32:
the gumbel-softmax hard weights are an exact one-hot (Sterbenz: for the
selected index y >= 0.5, so (1-y)+y == 1.0 exactly; for the other index
(0-y)+y == 0.0 exactly), and the relaxed-Bernoulli hard sample is exactly
0.0 or 1.0.  The whole forward therefore collapses to, per round:
pick op j = argmax(logits + gumbel); if the Bernoulli bit is 1, apply a
nearest-neighbor x-warp (shear-x or translate-x) with zero fill.

Both warps are per-row gathers along width whose source index is
round(w + const(h)), i.e. a per-row horizontal SHIFT with zero fill at the
invalid edges.  The composition of the two rounds is again a per-row shift
with a per-row valid interval.  The only remaining fp effect is the
straight-through magnitude perturbation x -> (x+m)-m (<= 1 ulp of ~2 per
round, ~2.4e-7 absolute), which we intentionally drop.

So the device kernel is pure data movement: for each group of rows with
identical (shift, valid interval) structure, one DRAM->DRAM DMA copy per
segment.  Invalid margins are NOT written at all: run_bass_kernel_spmd
guarantees pre-zeroed ExternalOutput buffers (the native path zeroes them
before run_neff; the axon/PJRT path donates freshly-zeroed buffers --
bass2jax.run_bass_via_pjrt documents that kernels which don't write every
element rely on this).  Skipping the margin writes avoids 7k+ sub-512B
descriptors whose read-modify-write amplification measured ~20% of kernel
time.  Data parallel over batch: 256/8 = 32 images per NeuronCore.
"""

import numpy as np

from concourse import bass
from concourse.bass_utils import run_bass_kernel_spmd
import concourse.mybir as mybir

B, H, W, C = 256, 224, 224, 3
N_CORES = 8
B_LOC = B // N_CORES
_EPS = 1e-6
_f32 = np.float32


def _round_decisions(sp_weights, sp_probs, sp_magnitudes, u_gumbel, u_logistic):
    """Mirror the reference's fp32 scalar math. Returns [(applied, j, m), ...]."""
    sp_weights = np.asarray(sp_weights, _f32)
    sp_probs = np.asarray(sp_probs, _f32)
    sp_magnitudes = np.asarray(sp_magnitudes, _f32)
    u_gumbel = np.asarray(u_gumbel, _f32)
    u_logistic = np.asarray(u_logistic, _f32)
    num_ops = sp_weights.shape[0]
    rounds = []
    for i in range(num_ops):
        u = np.clip(u_gumbel[i], _f32(_EPS), _f32(1.0 - _EPS))
        g = (-np.log(-np.log(u))).astype(_f32)
        z = (sp_weights[i] + g).astype(_f32)
        j = int(np.argmax(z))
        p = np.clip(sp_probs[i, j], _f32(_EPS), _f32(1.0 - _EPS))
        ul = np.clip(u_logistic[i, j], _f32(_EPS), _f32(1.0 - _EPS))
        l = (np.log(ul) - np.log1p(-ul)).astype(_f32)
        logit = ((np.log(p) - np.log1p(-p)).astype(_f32) + l).astype(_f32)
        y = _f32(1.0) / (_f32(1.0) + np.exp(-logit).astype(_f32))
        applied = bool(np.round(y) == 1.0)
        rounds.append((applied, j, sp_magnitudes[i, j]))
    return rounds


def _warp_maps(applied, j, m):
    """Per-round (xic[H,W] int32 clipped src index, valid[H,W] bool)."""
    ww = np.arange(W, dtype=_f32)[None, :]
    if not applied:
        xi = np.broadcast_to(np.arange(W, dtype=np.int32)[None, :], (H, W))
        return xi, np.ones((H, W), dtype=bool)
    if j == 0:  # ShearX: x_src = x + (0.6*m - 0.3) * y
        mag = _f32(_f32(0.6) * m - _f32(0.3))
        yy = np.arange(H, dtype=_f32)[:, None]
        x_src = (ww + (mag * yy).astype(_f32)).astype(_f32)
    else:  # TranslateX: x_src = x - (20*m - 10)
        pix = _f32(_f32(20.0) * m - _f32(10.0))
        x_src = np.broadcast_to((np.arange(W, dtype=_f32) - pix).astype(_f32)[None, :], (H, W))
    xi = np.round(x_src).astype(np.int32)
    valid = (xi >= 0) & (xi < W)
    xic = np.clip(xi, 0, W - 1)
    return xic, valid


def _composed_map(rounds):
    """Compose the per-round warps into (src[H,W] int, mask[H,W] bool)."""
    rows = np.arange(H)[:, None]
    src = np.broadcast_to(np.arange(W)[None, :], (H, W)).copy()
    mask = np.ones((H, W), dtype=bool)
    for applied, j, m in rounds:
        xic, valid = _warp_maps(applied, j, m)
        src = src[rows, xic]
        mask = mask[rows, xic] & valid
    return src, mask


def _row_groups(src, mask):
    """Decompose the map into row groups with identical segment structure.

    Returns list of (h0, h1, segs) where segs is a tuple of (lo, hi, s):
    out[:, h0:h1, lo:hi, :] = x[:, h0:h1, lo+s:hi+s, :], zero elsewhere.
    """
    per_row = []
    for h in range(H):
        segs = []
        w = 0
        mrow = mask[h]
        srow = src[h]
        while w < W:
            if not mrow[w]:
                w += 1
                continue
            s = int(srow[w]) - w
            w2 = w + 1
            while w2 < W and mrow[w2] and int(srow[w2]) - w2 == s:
                w2 += 1
            segs.append((w, w2, s))
            w = w2
        per_row.append(tuple(segs))
    groups = []
    h = 0
    while h < H:
        h2 = h + 1
        while h2 < H and per_row[h2] == per_row[h]:
            h2 += 1
        groups.append((h, h2, per_row[h]))
        h = h2
    return groups


def _gaps(segs):
    """Complement of the copy segments in [0, W)."""
    out = []
    prev = 0
    for lo, hi, _ in segs:
        if lo > prev:
            out.append((prev, lo))
        prev = hi
    if prev < W:
        out.append((prev, W))
    return out


def _build_program(groups):
    """One DRAM->DRAM DMA per copy segment, issued back-to-back on the SP
    HWDGE ring.  Zero margins are never written (output buffers arrive
    pre-zeroed, see module docstring).  DMA completions rotate over multiple
    semaphores so no single semaphore accumulates a large count (<= 120 DMAs
    x 16 incs each); the kernel waits on all of them at the end."""
    from contextlib import ExitStack

    dt = mybir.dt.float32
    nc = bass.Bass()
    xin = nc.declare_dram_parameter("xin", [B_LOC, H, W, C], dt, isOutput=False)
    out = nc.declare_dram_parameter("out", [B_LOC, H, W, C], dt, isOutput=True)

    jobs = []
    for h0, h1, segs in groups:
        for lo, hi, s in segs:
            jobs.append(((slice(None), slice(h0, h1), slice(lo, hi), slice(None)),
                         xin,
                         (slice(None), slice(h0, h1), slice(lo + s, hi + s), slice(None))))

    if not jobs:
        # fully-masked output (all zeros): nothing to copy, but emit one tiny
        # DMA into an internal scratch tensor so the program is non-empty.
        scratch = nc.dram_tensor("scratch", [1, C], dt)
        with nc.Block(no_gpsimd_drain=True) as block, nc.semaphore("dma_sem0") as sem:
            @block.sync
            def _(sync):
                sync.dma_start(out=scratch[:], in_=xin[0, 0, 0:1, :]).then_inc(sem, 16)
                sync.wait_ge(sem, 16)
        return nc

    chunk = 120
    max_sems = 64
    n_sems = max(1, min(max_sems, -(-len(jobs) // chunk)))
    batch = n_sems * chunk

    with ExitStack() as stack:
        # No GpSimd work is ever issued (HWDGE DMAs only), so skip GpSimd's
        # expensive dge_drain and use the sem-only end barrier.
        block = stack.enter_context(nc.Block(no_gpsimd_drain=True))
        sems = [stack.enter_context(nc.semaphore(f"dma_sem{i}")) for i in range(n_sems)]

        @block.sync
        def _(sync):
            target = [0] * n_sems
            for b0 in range(0, len(jobs), batch):
                bjobs = jobs[b0 : b0 + batch]
                for idx, (o_sl, src_t, i_sl) in enumerate(bjobs):
                    s = idx // chunk
                    sync.dma_start(out=out[o_sl], in_=src_t[i_sl]).then_inc(sems[s], 16)
                    target[s] += 16
                for s in range(n_sems):
                    if target[s]:
                        sync.wait_ge(sems[s], target[s])
                if b0 + batch < len(jobs):
                    # absurdly fragmented map: reset sems between mega-batches
                    for s in range(n_sems):
                        if target[s]:
                            sync.sem_clear(sems[s])
                            target[s] = 0

    return nc


_PROGRAM_CACHE = {}


def kernel(x, sp_weights, sp_probs, sp_magnitudes, u_gumbel, u_logistic):
    x = np.ascontiguousarray(np.asarray(x, dtype=np.float32))
    rounds = _round_decisions(sp_weights, sp_probs, sp_magnitudes, u_gumbel, u_logistic)
    src, mask = _composed_map(rounds)
    groups = _row_groups(src, mask)

    key = tuple(groups)
    nc = _PROGRAM_CACHE.get(key)
    if nc is None:
        nc = _build_program(groups)
        _PROGRAM_CACHE[key] = nc

    in_maps = [{"xin": x[i * B_LOC : (i + 1) * B_LOC]} for i in range(N_CORES)]
    res = run_bass_kernel_spmd(nc, in_maps, list(range(N_CORES)))
    return np.concatenate([res.results[i]["out"] for i in range(N_CORES)], axis=0)
